# revision 1
# baseline (speedup 1.0000x reference)
"""Trainium2 Bass kernel for nn_BasicNCAModel (neural cellular automaton).

Model (per step, 4 steps):
  y = concat([x, dwconv3x3(x, f1), dwconv3x3(x, f2)])   (reflect pad)
  dx = relu(y @ w1 + b1) @ w2
  x  = x + dx * (stoch > 0.5) * ch_mask

Strategy (pure data parallel, batch 16 -> 2 samples x 8 cores):
  - The depthwise convs + first dense layer fold into a 3x3 conv with
    effective weights W_eff[dy,dx] = diag(f1) @ w1[32:64] + diag(f2) @
    w1[64:96] (+ w1[0:32] at the center tap).
  - Layer 1 runs as fp8e4 DoubleRow matmuls. On TRN2 a matmul costs
    out-rows x 1 cycle regardless of K, so the win is K-packing: each
    DR matmul carries 2 K-planes per partition.  Per 512-px tile and
    output half only TWO matmuls:
      A: K=256 = 96 taps(plane1: dx=0 | plane2: dx=1) + 32 lo rows on
         partitions 96:128 (plane1 zero-weighted, plane2 = center tap
         applied to lo = x - fp8(x), recovering fp16-grade precision on
         the dominant identity tap).  The planes read the HI / DUP
         sections; DUP holds the same rows pre-shifted one column (DR
         plane strides must be even and non-overlapping on HW).
      B: K=192 = 96 taps(dx=2) + ones-rows carrying the bias.
  - Layer 2 is fp16 (fp8 h/w2 quantization fails the accuracy budget),
    M=32 per px-tile.  All 16 L2 matmuls of a band run as two
    weight-major batches sweeping the four 32-col PE array groups
    (tile_position=(0,32q)) so they execute concurrently.
  - Weights are x64 (fp8 subnormal avoidance); the 1/64 descale rides
    the relu's scale; mask is host-precomputed {0,1} fp8, replicated
    across channels by a stride-0 DMA.
  - relu+downcast splits between ACT (cols 0:ACT_COLS) and DVE.
  - dx for all 8 px-tiles of a 16-row band accumulates into one
    [128,1024] PSUM tile (4 quadrants x 2 col halves), so mask+residual
    run at full 128-partition width.
  - x state ping-pongs in DRAM as fp16 (exact carrier) + fp8 hi + fp8
    lo; hi/lo are regenerated on gpsimd each step from the fp16
    residual output.
"""

import numpy as np
import ml_dtypes
from contextlib import ExitStack

import concourse.bacc as bacc
import concourse.tile as tile
from concourse import mybir
from concourse.ap import AP
from concourse.bass_utils import run_bass_kernel_spmd

F32 = mybir.dt.float32
F16 = mybir.dt.float16
F8 = mybir.dt.float8e4
AF = mybir.ActivationFunctionType
OP = mybir.AluOpType
PM = mybir.MatmulPerfMode
NPF8 = ml_dtypes.float8_e4m3   # TRN-semantics e4m3 (max 240)

B, C, H, W = 16, 32, 256, 256
IMG = 3
NCORES = 8
BPC = B // NCORES          # samples per core
BR = 16                    # band rows
NB = H // BR               # bands per sample
RPT = 2                    # rows per px tile
TPB = BR // RPT            # px tiles per band = 8
NSTEP = 4
WP = W + 2                 # padded row length

HI_OFF = 0                 # xt8 layout: 16 rows / 16 dup rows / 2 ones rows
DUP_OFF = BR * WP          # same rows shifted one column left (tap dx=1 and
                           # the lo plane read here; DR plane strides must be
                           # even and the regions non-overlapping on HW)
ONES_OFF = 2 * BR * WP
XT_FREE = ONES_OFF + 2 * WP    # 34*WP

ACT_COLS = 736             # relu columns handled by ACT; rest by DVE
DEBUG_DUMP = False         # dump band-0 intermediates as extra outputs
S1 = 64.0                  # layer-1 weight scale
DESC = 1.0 / 64.0          # descale applied in the relu


def _hi_segments(r0: int, g: int):
    """(src_row, dst_row, n) contiguous segments for hi group g (dy=g-1)."""
    rows = [r0 - 1 + g + i for i in range(BR)]
    refl = [(-r if r < 0 else (2 * (H - 1) - r if r > H - 1 else r))
            for r in rows]
    segs, i = [], 0
    while i < BR:
        j = i + 1
        while j < BR and refl[j] == refl[i] + (j - i):
            j += 1
        segs.append((refl[i], i, j - i))
        i = j
    return segs


def _build():
    nc = bacc.Bacc("TRN2", target_bir_lowering=False, debug=False,
                   num_devices=NCORES)
    PL = C * H * WP  # per-sample plane elements
    x16i = nc.dram_tensor("x16i", [BPC, C, H, WP], F16, kind="ExternalInput").ap()
    x8hi_i = nc.dram_tensor("x8hi", [BPC, C, H, WP], F8, kind="ExternalInput").ap()
    x8lo_i = nc.dram_tensor("x8lo", [BPC, C, H, WP], F8, kind="ExternalInput").ap()
    mask8 = nc.dram_tensor("mask8", [NSTEP, BPC, H, W], F8, kind="ExternalInput").ap()
    wa_d = nc.dram_tensor("wa", [2, 128, 256], F8, kind="ExternalInput").ap()
    wb_d = nc.dram_tensor("wb", [2, 128, 256], F8, kind="ExternalInput").ap()
    w2a_d = nc.dram_tensor("w2a", [128, 32], F16, kind="ExternalInput").ap()
    w2b_d = nc.dram_tensor("w2b", [128, 32], F16, kind="ExternalInput").ap()
    yout = nc.dram_tensor("y", [BPC, C, H, WP], F16, kind="ExternalOutput").ap()
    if DEBUG_DUMP:
        dbg_xt = nc.dram_tensor("dbg_xt", [128, XT_FREE], F8, kind="ExternalOutput").ap()
        dbg_mb = nc.dram_tensor("dbg_mb", [128, 1024], F8, kind="ExternalOutput").ap()
        dbg_xcq = nc.dram_tensor("dbg_xcq", [128, 4 * WP], F16, kind="ExternalOutput").ap()
        dbg_hs = nc.dram_tensor("dbg_hs", [128, 1024], F16, kind="ExternalOutput").ap()
        dbg_dx = nc.dram_tensor("dbg_dx", [128, 1024], F32, kind="ExternalOutput").ap()
        dbg_dxm = nc.dram_tensor("dbg_dxm", [128, 1024], F16, kind="ExternalOutput").ap()
        dbg_xn = nc.dram_tensor("dbg_xn", [128, 4 * WP], F16, kind="ExternalOutput").ap()

    with tile.TileContext(nc) as tc, ExitStack() as ctx:
        dram = ctx.enter_context(tc.tile_pool(name="dram", bufs=1, space="DRAM"))
        x16A = dram.tile([BPC, C, H, WP], F16, name="x16A")
        x16B = dram.tile([BPC, C, H, WP], F16, name="x16B")
        x8hA = dram.tile([BPC, C, H, WP], F8, name="x8hA")
        x8hB = dram.tile([BPC, C, H, WP], F8, name="x8hB")
        x8lA = dram.tile([BPC, C, H, WP], F8, name="x8lA")
        x8lB = dram.tile([BPC, C, H, WP], F8, name="x8lB")

        wpool = ctx.enter_context(tc.tile_pool(name="wpool", bufs=1))
        wAt = [wpool.tile([128, 256], F8, name=f"wA{h}") for h in range(2)]
        wBt = [wpool.tile([128, 256], F8, name=f"wB{h}") for h in range(2)]
        w2at = wpool.tile([128, 32], F16, name="w2at")
        w2bt = wpool.tile([128, 32], F16, name="w2bt")
        zeros = wpool.tile([128, 1024], F16, name="zeros")
        for h in range(2):
            nc.sync.dma_start(wAt[h][:], wa_d[h])
            nc.sync.dma_start(wBt[h][:], wb_d[h])
        nc.sync.dma_start(w2at[:], w2a_d)
        nc.sync.dma_start(w2bt[:], w2b_d)
        nc.vector.memset(zeros[:], 0.0)

        xt_pool = ctx.enter_context(tc.tile_pool(name="xt", bufs=3))
        xcq_pool = ctx.enter_context(tc.tile_pool(name="xcq", bufs=2))
        mb_pool = ctx.enter_context(tc.tile_pool(name="mb", bufs=2))
        hs_pool = ctx.enter_context(tc.tile_pool(name="hs", bufs=10))
        dxm_pool = ctx.enter_context(tc.tile_pool(name="dxm", bufs=2))
        xn_pool = ctx.enter_context(tc.tile_pool(name="xn", bufs=2))
        hi8_pool = ctx.enter_context(tc.tile_pool(name="hi8", bufs=2))
        lo8_pool = ctx.enter_context(tc.tile_pool(name="lo8", bufs=2))
        hp_pool = ctx.enter_context(tc.tile_pool(name="hp", bufs=3, space="PSUM"))
        dx_pool = ctx.enter_context(tc.tile_pool(name="dxp", bufs=1, space="PSUM"))

        ab16, ab8h, ab8l = [x16A[:], x16B[:]], [x8hA[:], x8hB[:]], [x8lA[:], x8lB[:]]
        s16 = [x16i] + [ab16[i % 2] for i in range(NSTEP - 1)]
        d16 = [ab16[i % 2] for i in range(NSTEP - 1)] + [yout]
        s8h = [x8hi_i] + [ab8h[i % 2] for i in range(NSTEP - 1)]
        d8h = [ab8h[i % 2] for i in range(NSTEP - 1)] + [None]
        s8l = [x8lo_i] + [ab8l[i % 2] for i in range(NSTEP - 1)]
        d8l = [ab8l[i % 2] for i in range(NSTEP - 1)] + [None]

        def wv(t, n=128):
            return t[0:n, :].rearrange("p (two m) -> p two m", two=2)

        for st in range(NSTEP):
            src16, dst16 = s16[st], d16[st]
            srch, dsth = s8h[st], d8h[st]
            srcl, dstl = s8l[st], d8l[st]
            last = st == NSTEP - 1
            for s in range(BPC):
                for b in range(NB):
                    r0 = b * BR
                    # ---- xt8: hi (3 dy groups + lo rows) and its dup ----
                    xt8 = xt_pool.tile([128, XT_FREE], F8, name="xt8")
                    th = xt8[:].tensor
                    for dup, sh in ((HI_OFF, 0), (DUP_OFF, 1)):
                        if 1 <= b <= NB - 2:
                            nc.sync.dma_start(
                                AP(th, dup, [[XT_FREE, 96], [1, BR * WP]]),
                                AP(srch.tensor,
                                   srch.offset + s * PL + (r0 - 1) * WP + sh,
                                   [[WP, 3], [H * WP, 32], [1, BR * WP]]))
                        else:
                            for g in range(3):
                                for (sr, dr, n) in _hi_segments(r0, g):
                                    ln = n * WP - (sh if sr + n == H else 0)
                                    nc.sync.dma_start(
                                        AP(th, g * 32 * XT_FREE + dup + dr * WP,
                                           [[XT_FREE, 32], [1, ln]]),
                                        AP(srch.tensor,
                                           srch.offset + s * PL + sr * WP + sh,
                                           [[H * WP, 32], [1, ln]]))
                        if sh == 0:
                            # partitions 96:128 HI: hi-center rows shifted -1
                            # (B plane1... read at the dx2 offset = center
                            # window, weighted by the center-tap residual)
                            d0 = 1 if r0 == 0 else 0
                            nc.sync.dma_start(
                                AP(th, 96 * XT_FREE + d0,
                                   [[XT_FREE, 32], [1, BR * WP - d0]]),
                                AP(srch.tensor,
                                   srch.offset + s * PL + r0 * WP - 1 + d0,
                                   [[H * WP, 32], [1, BR * WP - d0]]))
                            if r0 == 0:
                                nc.gpsimd.memset(xt8[96:128, 0:1], 0.0)
                        else:
                            # partitions 96:128 DUP: lo rows shifted +1
                            # (A plane 2 = center tap applied to lo)
                            ln = BR * WP - (1 if r0 + BR == H else 0)
                            nc.sync.dma_start(
                                AP(th, 96 * XT_FREE + dup,
                                   [[XT_FREE, 32], [1, ln]]),
                                AP(srcl.tensor,
                                   srcl.offset + s * PL + r0 * WP + 1,
                                   [[H * WP, 32], [1, ln]]))
                    nc.gpsimd.memset(xt8[:, ONES_OFF:XT_FREE], 1.0)
                    # reflect-column fix on the hi rows
                    xtv = xt8[:].rearrange("p (r c) -> p r c", c=WP)
                    nc.vector.tensor_copy(xtv[0:96, 0:BR, 0:1],
                                          xtv[0:96, 0:BR, 2:3])
                    nc.vector.tensor_copy(xtv[0:96, 0:BR, WP - 1:WP],
                                          xtv[0:96, 0:BR, WP - 3:WP - 2])

                    # ---- exact fp16 x (quad layout) + mask ----
                    xcq = xcq_pool.tile([128, 4 * WP], F16, name="xcq")
                    nc.sync.dma_start(
                        AP(xcq[:].tensor, 0, [[4 * WP, 128], [1, 4 * WP]]),
                        AP(src16.tensor, src16.offset + s * PL + r0 * WP,
                           [[4 * WP, 4], [H * WP, 32], [1, 4 * WP]]))
                    mb = mb_pool.tile([128, 1024], F8, name="mb")
                    nc.sync.dma_start(
                        AP(mb[:].tensor, 0, [[1024, 128], [1, 1024]]),
                        AP(mask8.tensor,
                           mask8.offset + ((st * BPC + s) * H + r0) * W,
                           [[4 * W, 4], [0, 32], [1, 4 * W]]))

                    # ---- layer 1: weight-major over px-tile pairs ----
                    dxo = dx_pool.tile([128, 1024], F32, name="dxo")
                    hs = [None] * TPB
                    hp = [None] * TPB

                    def emit_l1(t, h, kind):
                        r = RPT * t
                        if kind == 0:
                            lhs, rhs = wv(wAt[h]), AP(
                                th, HI_OFF + r * WP,
                                [[XT_FREE, 128], [DUP_OFF, 2],
                                 [WP, RPT], [1, W]])
                        else:
                            lhs, rhs = wv(wBt[h]), AP(
                                th, HI_OFF + r * WP + 2,
                                [[XT_FREE, 128],
                                 [ONES_OFF - (r * WP + 2), 2],
                                 [WP, RPT], [1, W]])
                        nc.tensor.matmul(
                            hp[t][:, h * 512:(h + 1) * 512],
                            lhs, rhs, start=(kind == 0), stop=(kind == 1),
                            perf_mode=PM.DoubleRow, skip_group_check=True)

                    for p in range(TPB // 2):
                        pair = (2 * p, 2 * p + 1)
                        for t in pair:
                            hp[t] = hp_pool.tile([128, 1024], F32, name="hp")
                        for h in range(2):
                            for kind in range(2):
                                for t in pair:
                                    emit_l1(t, h, kind)
                        for t in pair:
                            hs[t] = hs_pool.tile([128, 1024], F16, name="hs")
                            nc.scalar.activation(hs[t][:, 0:ACT_COLS],
                                                 hp[t][:, 0:ACT_COLS],
                                                 AF.Relu, scale=DESC)
                            nc.vector.scalar_tensor_tensor(
                                hs[t][:, ACT_COLS:1024],
                                hp[t][:, ACT_COLS:1024], DESC,
                                zeros[:, ACT_COLS:1024],
                                op0=OP.mult, op1=OP.max)

                    # ---- layer 2: two weight-major batches; the four
                    # 32-col PE array groups run concurrently ----
                    order = [0, 2, 4, 6, 1, 3, 5, 7]
                    for t in order:
                        q, hh = t // 2, t % 2
                        nc.tensor.matmul(
                            dxo[32 * q:32 * q + 32, 512 * hh:512 * hh + 512],
                            w2at[:], hs[t][:, 0:512],
                            start=True, stop=False, skip_group_check=True,
                            tile_position=(0, 32 * q))
                    for t in order:
                        q, hh = t // 2, t % 2
                        nc.tensor.matmul(
                            dxo[32 * q:32 * q + 32, 512 * hh:512 * hh + 512],
                            w2bt[:], hs[t][:, 512:1024],
                            start=False, stop=True, skip_group_check=True,
                            tile_position=(0, 32 * q))

                    dbg = DEBUG_DUMP and st == 0 and s == 0 and b == 0
                    if dbg:
                        nc.sync.dma_start(dbg_xt, xt8[:])
                        nc.sync.dma_start(dbg_mb, mb[:])
                        nc.sync.dma_start(dbg_xcq, xcq[:])
                        nc.sync.dma_start(dbg_hs, hs[0][:])
                        dxc = dxm_pool.tile([128, 1024], F32, name="dxc")
                        nc.vector.tensor_copy(dxc[:], dxo[:])
                        nc.sync.dma_start(dbg_dx, dxc[:])

                    # ---- mask * dx, residual add (quad layout) ----
                    dxm = dxm_pool.tile([128, 1024], F16, name="dxm")
                    nc.vector.tensor_mul(dxm[:], dxo[:], mb[:])
                    xn = xn_pool.tile([128, 4 * WP], F16, name="xn")
                    xnv = xn[:].rearrange("p (b c) -> p b c", c=WP)
                    nc.gpsimd.memset(xnv[:, :, 0:WP:WP - 1], 0.0)
                    nc.vector.scalar_tensor_tensor(
                        xnv[:, :, 1:W + 1],
                        dxm[:].rearrange("p (b c) -> p b c", c=W), 1.0,
                        xcq[:].rearrange("p (b c) -> p b c", c=WP)[:, :, 1:W + 1],
                        op0=OP.mult, op1=OP.add)
                    if dbg:
                        nc.sync.dma_start(dbg_dxm, dxm[:])
                        nc.sync.dma_start(dbg_xn, xn[:])

                    # ---- stores + next-step hi/lo generation ----
                    dst_ap = AP(dst16.tensor, dst16.offset + s * PL + r0 * WP,
                                [[4 * WP, 4], [H * WP, 32], [1, 4 * WP]])
                    nc.sync.dma_start(dst_ap, xn[:])
                    if not last:
                        hi8 = hi8_pool.tile([128, 4 * WP], F8, name="hi8")
                        nc.gpsimd.tensor_copy(hi8[:], xn[:])
                        lo8 = lo8_pool.tile([128, 4 * WP], F8, name="lo8")
                        nc.gpsimd.tensor_sub(lo8[:], xn[:], hi8[:])
                        nc.sync.dma_start(
                            AP(dsth.tensor, dsth.offset + s * PL + r0 * WP,
                               [[4 * WP, 4], [H * WP, 32], [1, 4 * WP]]), hi8[:])
                        nc.sync.dma_start(
                            AP(dstl.tensor, dstl.offset + s * PL + r0 * WP,
                               [[4 * WP, 4], [H * WP, 32], [1, 4 * WP]]), lo8[:])
    nc.compile()
    return nc


_NC_CACHE = None


def _get_nc():
    global _NC_CACHE
    if _NC_CACHE is None:
        _NC_CACHE = _build()
    return _NC_CACHE


def _q8(x):
    return np.asarray(x, np.float32).astype(NPF8)


def _make_in_maps(x, f1, f2, w1, b1, w2, stoch):
    f1 = np.asarray(f1, np.float64)[:, :, 0, :]   # [3,3,32]
    f2 = np.asarray(f2, np.float64)[:, :, 0, :]
    w1 = np.asarray(w1, np.float64)               # [96,256]
    b1 = np.asarray(b1, np.float64)               # [256]
    w2 = np.asarray(w2, np.float64).copy()        # [256,32]
    w2[:, :IMG] = 0.0                             # ch_mask folded into w2

    # W_eff[dy,dx][c,:] = f1[dy,dx,c]*w1[32+c,:] + f2[dy,dx,c]*w1[64+c,:]
    weff = (f1[:, :, :, None] * w1[None, None, 32:64, :]
            + f2[:, :, :, None] * w1[None, None, 64:96, :])   # [3,3,32,256]
    weff[1, 1] += w1[0:32, :]
    weff = weff.astype(np.float32)

    wc_q = _q8(S1 * weff[1, 1]).astype(np.float32) / S1   # center tap as fp8
    wres = (weff[1, 1] - wc_q).astype(np.float32)          # its residual
    wa = np.zeros((2, 128, 2, 128), np.float32)
    wb = np.zeros((2, 128, 2, 128), np.float32)
    for h in range(2):
        cols = slice(128 * h, 128 * (h + 1))
        for g in range(3):
            rows = slice(32 * g, 32 * (g + 1))
            wa[h, rows, 0, :] = S1 * weff[g, 0, :, cols]
            wa[h, rows, 1, :] = S1 * weff[g, 1, :, cols]
            wb[h, rows, 0, :] = S1 * weff[g, 2, :, cols]
        wa[h, 96:128, 1, :] = S1 * weff[1, 1, :, cols]   # lo through center w
        wb[h, 96:128, 0, :] = S1 * wres[:, cols]         # wres on hi-center
        wb[h, 0, 1, :] = S1 * np.asarray(b1, np.float32)[cols]
    wa = _q8(wa.reshape(2, 128, 256))
    wb = _q8(wb.reshape(2, 128, 256))
    w2a = np.asarray(w2[0:128, :], np.float32).astype(np.float16)
    w2b = np.asarray(w2[128:256, :], np.float32).astype(np.float16)

    x = np.asarray(x, np.float32)
    stoch = np.asarray(stoch, np.float32)
    in_maps = []
    for i in range(NCORES):
        xi = np.transpose(x[i * BPC:(i + 1) * BPC], (0, 3, 1, 2))  # [2,32,H,W]
        xpad = np.zeros((BPC, C, H, WP), np.float32)
        xpad[:, :, :, 1:W + 1] = xi
        x16 = xpad.astype(np.float16)
        x8h = x16.astype(np.float32).astype(NPF8)
        x8l = (x16.astype(np.float32) - x8h.astype(np.float32)).astype(NPF8)
        m8 = _q8((stoch[:, i * BPC:(i + 1) * BPC, :, :, 0] > 0.5))
        in_maps.append({"x16i": x16, "x8hi": x8h, "x8lo": x8l, "mask8": m8,
                        "wa": wa, "wb": wb, "w2a": w2a, "w2b": w2b})
    return in_maps


def kernel(x, f1, f2, w1, b1, w2, stoch, steps):
    assert int(steps) == NSTEP, f"kernel compiled for {NSTEP} steps, got {steps}"
    nc = _get_nc()
    in_maps = _make_in_maps(x, f1, f2, w1, b1, w2, stoch)
    res = run_bass_kernel_spmd(nc, in_maps, core_ids=list(range(NCORES)))
    outs = []
    for i in range(NCORES):
        yi = res.results[i]["y"].astype(np.float32)[:, :, :, 1:W + 1]
        outs.append(np.transpose(yi, (0, 2, 3, 1)))   # -> [2,256,256,32]
    return np.ascontiguousarray(np.concatenate(outs, axis=0)).astype(np.float32)



# revision 8
# speedup vs baseline: 1.2158x; 1.2158x over previous
"""Trainium2 Bass kernel for nn_BasicNCAModel (neural cellular automaton).

Model (per step, 4 steps):
  y = concat([x, dwconv3x3(x, f1), dwconv3x3(x, f2)])   (reflect pad)
  dx = relu(y @ w1 + b1) @ w2
  x  = x + dx * (stoch > 0.5) * ch_mask

Strategy (pure data parallel, batch 16 -> 2 samples x 8 cores):
  - The depthwise convs + first dense layer fold into a 3x3 conv with
    effective weights W_eff[dy,dx] = diag(f1) @ w1[32:64] + diag(f2) @
    w1[64:96] (+ w1[0:32] at the center tap).
  - Layer 1 runs as fp8e4 DoubleRow matmuls. On TRN2 a matmul costs
    out-rows x 1 cycle regardless of K, so the win is K-packing: each
    DR matmul carries 2 K-planes per partition.  Per 512-px tile and
    output half only TWO matmuls:
      A: K=256 = 96 taps(plane1: dx=0 | plane2: dx=1) + 32 lo rows on
         partitions 96:128 (plane1 zero-weighted, plane2 = center tap
         applied to lo = x - fp8(x), recovering fp16-grade precision on
         the dominant identity tap).  The planes read the HI / DUP
         sections; DUP holds the same rows pre-shifted one column (DR
         plane strides must be even and non-overlapping on HW).
      B: K=192 = 96 taps(dx=2) + ones-rows carrying the bias.
  - Layer 2 is fp16 (fp8 h/w2 quantization fails the accuracy budget),
    M=32 per px-tile.  All 16 L2 matmuls of a band run as two
    weight-major batches sweeping the four 32-col PE array groups
    (tile_position=(0,32q)) so they execute concurrently.
  - Weights are x64 (fp8 subnormal avoidance); the 1/64 descale rides
    the relu's scale; mask is host-precomputed {0,1} fp8, replicated
    across channels by a stride-0 DMA.
  - relu+downcast splits between ACT (cols 0:ACT_COLS) and DVE.
  - dx for all 8 px-tiles of a 16-row band accumulates into one
    [128,1024] PSUM tile (4 quadrants x 2 col halves), so mask+residual
    run at full 128-partition width.
  - x state ping-pongs in DRAM as fp16 (exact carrier) + fp8 hi + fp8
    lo; hi/lo are regenerated on gpsimd each step from the fp16
    residual output.
"""

import numpy as np
import ml_dtypes
from contextlib import ExitStack

import concourse.bacc as bacc
import concourse.tile as tile
from concourse import mybir
from concourse.ap import AP
from concourse.bass_utils import run_bass_kernel_spmd

F32 = mybir.dt.float32
F16 = mybir.dt.float16
F8 = mybir.dt.float8e4
AF = mybir.ActivationFunctionType
OP = mybir.AluOpType
PM = mybir.MatmulPerfMode
NPF8 = ml_dtypes.float8_e4m3   # TRN-semantics e4m3 (max 240)

B, C, H, W = 16, 32, 256, 256
IMG = 3
NCORES = 8
BPC = B // NCORES          # samples per core
BR = 16                    # band rows
NB = H // BR               # bands per sample
RPT = 2                    # rows per px tile
TPB = BR // RPT            # px tiles per band = 8
NSTEP = 4
WP = W + 2                 # padded row length

HI_OFF = 0                 # xt8 layout: 16 rows / 16 dup rows / 2 ones rows
DUP_OFF = BR * WP          # same rows shifted one column left (tap dx=1 and
                           # the lo plane read here; DR plane strides must be
                           # even and the regions non-overlapping on HW)
ONES_OFF = 2 * BR * WP
XT_FREE = ONES_OFF + 2 * WP    # 34*WP

ACT_COLS = 736             # relu columns handled by ACT; rest by DVE
DEBUG_DUMP = False         # dump band-0 intermediates as extra outputs
S1 = 64.0                  # layer-1 weight scale
DESC = 1.0 / 64.0          # descale applied in the relu


def _hi_segments(r0: int, g: int):
    """(src_row, dst_row, n) contiguous segments for hi group g (dy=g-1)."""
    rows = [r0 - 1 + g + i for i in range(BR)]
    refl = [(-r if r < 0 else (2 * (H - 1) - r if r > H - 1 else r))
            for r in rows]
    segs, i = [], 0
    while i < BR:
        j = i + 1
        while j < BR and refl[j] == refl[i] + (j - i):
            j += 1
        segs.append((refl[i], i, j - i))
        i = j
    return segs


def _build():
    nc = bacc.Bacc("TRN2", target_bir_lowering=False, debug=False,
                   num_devices=NCORES)
    PL = C * H * WP  # per-sample plane elements
    x16i = nc.dram_tensor("x16i", [BPC, C, H, WP], F16, kind="ExternalInput").ap()
    x8hi_i = nc.dram_tensor("x8hi", [BPC, C, H, WP], F8, kind="ExternalInput").ap()
    x8lo_i = nc.dram_tensor("x8lo", [BPC, C, H, WP], F8, kind="ExternalInput").ap()
    mask8 = nc.dram_tensor("mask8", [NSTEP, BPC, H, W], F8, kind="ExternalInput").ap()
    wa_d = nc.dram_tensor("wa", [2, 128, 256], F8, kind="ExternalInput").ap()
    wb_d = nc.dram_tensor("wb", [2, 128, 256], F8, kind="ExternalInput").ap()
    w2a_d = nc.dram_tensor("w2a", [128, 32], F16, kind="ExternalInput").ap()
    w2b_d = nc.dram_tensor("w2b", [128, 32], F16, kind="ExternalInput").ap()
    yout = nc.dram_tensor("y", [BPC, C, H, WP], F16, kind="ExternalOutput").ap()
    if DEBUG_DUMP:
        dbg_xt = nc.dram_tensor("dbg_xt", [128, XT_FREE], F8, kind="ExternalOutput").ap()
        dbg_mb = nc.dram_tensor("dbg_mb", [128, 1024], F8, kind="ExternalOutput").ap()
        dbg_xcq = nc.dram_tensor("dbg_xcq", [128, 4 * WP], F16, kind="ExternalOutput").ap()
        dbg_hs = nc.dram_tensor("dbg_hs", [128, 1024], F16, kind="ExternalOutput").ap()
        dbg_dx = nc.dram_tensor("dbg_dx", [128, 1024], F32, kind="ExternalOutput").ap()
        dbg_dxm = nc.dram_tensor("dbg_dxm", [128, 1024], F16, kind="ExternalOutput").ap()
        dbg_xn = nc.dram_tensor("dbg_xn", [128, 4 * WP], F16, kind="ExternalOutput").ap()

    with tile.TileContext(nc) as tc, ExitStack() as ctx:
        dram = ctx.enter_context(tc.tile_pool(name="dram", bufs=1, space="DRAM"))
        x16A = dram.tile([BPC, C, H, WP], F16, name="x16A")
        x16B = dram.tile([BPC, C, H, WP], F16, name="x16B")
        x8hA = dram.tile([BPC, C, H, WP], F8, name="x8hA")
        x8hB = dram.tile([BPC, C, H, WP], F8, name="x8hB")
        x8lA = dram.tile([BPC, C, H, WP], F8, name="x8lA")
        x8lB = dram.tile([BPC, C, H, WP], F8, name="x8lB")

        wpool = ctx.enter_context(tc.tile_pool(name="wpool", bufs=1))
        wAt = [wpool.tile([128, 256], F8, name=f"wA{h}") for h in range(2)]
        wBt = [wpool.tile([128, 256], F8, name=f"wB{h}") for h in range(2)]
        w2at = wpool.tile([128, 32], F16, name="w2at")
        w2bt = wpool.tile([128, 32], F16, name="w2bt")
        zeros = wpool.tile([128, 1024], F16, name="zeros")
        for h in range(2):
            nc.sync.dma_start(wAt[h][:], wa_d[h])
            nc.sync.dma_start(wBt[h][:], wb_d[h])
        nc.sync.dma_start(w2at[:], w2a_d)
        nc.sync.dma_start(w2bt[:], w2b_d)
        nc.vector.memset(zeros[:], 0.0)

        xt_pool = ctx.enter_context(tc.tile_pool(name="xt", bufs=3))
        xcq_pool = ctx.enter_context(tc.tile_pool(name="xcq", bufs=2))
        mb_pool = ctx.enter_context(tc.tile_pool(name="mb", bufs=2))
        hs_pool = ctx.enter_context(tc.tile_pool(name="hs", bufs=10))
        dxm_pool = ctx.enter_context(tc.tile_pool(name="dxm", bufs=2))
        xn_pool = ctx.enter_context(tc.tile_pool(name="xn", bufs=2))
        hi8_pool = ctx.enter_context(tc.tile_pool(name="hi8", bufs=2))
        lo8_pool = ctx.enter_context(tc.tile_pool(name="lo8", bufs=2))
        hp_pool = ctx.enter_context(tc.tile_pool(name="hp", bufs=3, space="PSUM"))
        dx_pool = ctx.enter_context(tc.tile_pool(name="dxp", bufs=1, space="PSUM"))

        ab16, ab8h, ab8l = [x16A[:], x16B[:]], [x8hA[:], x8hB[:]], [x8lA[:], x8lB[:]]
        s16 = [x16i] + [ab16[i % 2] for i in range(NSTEP - 1)]
        d16 = [ab16[i % 2] for i in range(NSTEP - 1)] + [yout]
        s8h = [x8hi_i] + [ab8h[i % 2] for i in range(NSTEP - 1)]
        d8h = [ab8h[i % 2] for i in range(NSTEP - 1)] + [None]
        s8l = [x8lo_i] + [ab8l[i % 2] for i in range(NSTEP - 1)]
        d8l = [ab8l[i % 2] for i in range(NSTEP - 1)] + [None]

        def wv(t, n=128):
            return t[0:n, :].rearrange("p (two m) -> p two m", two=2)

        for st in range(NSTEP):
            src16, dst16 = s16[st], d16[st]
            srch, dsth = s8h[st], d8h[st]
            srcl, dstl = s8l[st], d8l[st]
            last = st == NSTEP - 1
            for s in range(BPC):
                for b in range(NB):
                    r0 = b * BR
                    # ---- xt8: hi (3 dy groups + lo rows) and its dup ----
                    # One DMA per (dy group, hi/dup): the DRAM-side AP then
                    # has the 32-channel dim outermost, and HWDGE assigns
                    # descriptors to SDMA engines by DRAM-side outer-dim
                    # index (mod 16) -> all 16 engines, not 3-4.  SBUF-side
                    # APs must keep partition stepping in dim 0 only.
                    xt8 = xt_pool.tile([128, XT_FREE], F8, name="xt8")
                    th = xt8[:].tensor
                    for g in range(3):
                        for (sr, dr, n) in _hi_segments(r0, g):
                            for dup, sh in ((HI_OFF, 0), (DUP_OFF, 1)):
                                ln = n * WP - (sh if sr + n == H else 0)
                                nc.sync.dma_start(
                                    AP(th, g * 32 * XT_FREE + dup + dr * WP,
                                       [[XT_FREE, 32], [1, ln]]),
                                    AP(srch.tensor,
                                       srch.offset + s * PL + sr * WP + sh,
                                       [[H * WP, 32], [1, ln]]))
                    # partitions 96:128 HI: hi-center rows shifted -1
                    # (B plane1... read at the dx2 offset = center
                    # window, weighted by the center-tap residual)
                    d0 = 1 if r0 == 0 else 0
                    nc.sync.dma_start(
                        AP(th, 96 * XT_FREE + d0,
                           [[XT_FREE, 32], [1, BR * WP - d0]]),
                        AP(srch.tensor,
                           srch.offset + s * PL + r0 * WP - 1 + d0,
                           [[H * WP, 32], [1, BR * WP - d0]]))
                    if r0 == 0:
                        nc.gpsimd.memset(xt8[96:128, 0:1], 0.0)
                    # partitions 96:128 DUP: lo rows shifted +1
                    # (A plane 2 = center tap applied to lo)
                    ln = BR * WP - (1 if r0 + BR == H else 0)
                    nc.sync.dma_start(
                        AP(th, 96 * XT_FREE + DUP_OFF,
                           [[XT_FREE, 32], [1, ln]]),
                        AP(srcl.tensor,
                           srcl.offset + s * PL + r0 * WP + 1,
                           [[H * WP, 32], [1, ln]]))
                    nc.gpsimd.memset(xt8[:, ONES_OFF:XT_FREE], 1.0)
                    # reflect-column fix on the hi rows
                    xtv = xt8[:].rearrange("p (r c) -> p r c", c=WP)
                    nc.vector.tensor_copy(xtv[0:96, 0:BR, 0:1],
                                          xtv[0:96, 0:BR, 2:3])
                    nc.vector.tensor_copy(xtv[0:96, 0:BR, WP - 1:WP],
                                          xtv[0:96, 0:BR, WP - 3:WP - 2])

                    # ---- exact fp16 x (quad layout) + mask ----
                    # one DMA per quad -> DRAM-side outer dim = 32 channels
                    xcq = xcq_pool.tile([128, 4 * WP], F16, name="xcq")
                    for q in range(4):
                        nc.sync.dma_start(
                            AP(xcq[:].tensor, 32 * q * 4 * WP,
                               [[4 * WP, 32], [1, 4 * WP]]),
                            AP(src16.tensor,
                               src16.offset + s * PL + (r0 + 4 * q) * WP,
                               [[H * WP, 32], [1, 4 * WP]]))
                    mb = mb_pool.tile([128, 1024], F8, name="mb")
                    moff = mask8.offset + ((st * BPC + s) * H + r0) * W
                    for q in range(4):
                        nc.scalar.dma_start(
                            AP(mb[:].tensor, 32 * q * 1024,
                               [[1024, 32], [1, 1024]]),
                            AP(mask8.tensor, moff + q * 4 * W,
                               [[0, 32], [1, 4 * W]]))

                    # ---- layer 1: weight-major over px-tile pairs ----
                    dxo = dx_pool.tile([128, 1024], F32, name="dxo")
                    hs = [None] * TPB
                    hp = [None] * TPB

                    def emit_l1(t, h, kind):
                        r = RPT * t
                        if kind == 0:
                            lhs, rhs = wv(wAt[h]), AP(
                                th, HI_OFF + r * WP,
                                [[XT_FREE, 128], [DUP_OFF, 2],
                                 [WP, RPT], [1, W]])
                        else:
                            lhs, rhs = wv(wBt[h]), AP(
                                th, HI_OFF + r * WP + 2,
                                [[XT_FREE, 128],
                                 [ONES_OFF - (r * WP + 2), 2],
                                 [WP, RPT], [1, W]])
                        nc.tensor.matmul(
                            hp[t][:, h * 512:(h + 1) * 512],
                            lhs, rhs, start=(kind == 0), stop=(kind == 1),
                            perf_mode=PM.DoubleRow, skip_group_check=True)

                    for p in range(TPB // 2):
                        pair = (2 * p, 2 * p + 1)
                        for t in pair:
                            hp[t] = hp_pool.tile([128, 1024], F32, name="hp")
                        for h in range(2):
                            for kind in range(2):
                                for t in pair:
                                    emit_l1(t, h, kind)
                        for t in pair:
                            hs[t] = hs_pool.tile([128, 1024], F16, name="hs")
                            nc.scalar.activation(hs[t][:, 0:ACT_COLS],
                                                 hp[t][:, 0:ACT_COLS],
                                                 AF.Relu, scale=DESC)
                            nc.vector.scalar_tensor_tensor(
                                hs[t][:, ACT_COLS:1024],
                                hp[t][:, ACT_COLS:1024], DESC,
                                zeros[:, ACT_COLS:1024],
                                op0=OP.mult, op1=OP.max)

                    # ---- layer 2: two weight-major batches; the four
                    # 32-col PE array groups run concurrently ----
                    order = [0, 2, 4, 6, 1, 3, 5, 7]
                    for t in order:
                        q, hh = t // 2, t % 2
                        nc.tensor.matmul(
                            dxo[32 * q:32 * q + 32, 512 * hh:512 * hh + 512],
                            w2at[:], hs[t][:, 0:512],
                            start=True, stop=False, skip_group_check=True,
                            tile_position=(0, 32 * q))
                    for t in order:
                        q, hh = t // 2, t % 2
                        nc.tensor.matmul(
                            dxo[32 * q:32 * q + 32, 512 * hh:512 * hh + 512],
                            w2bt[:], hs[t][:, 512:1024],
                            start=False, stop=True, skip_group_check=True,
                            tile_position=(0, 32 * q))

                    dbg = DEBUG_DUMP and st == 0 and s == 0 and b == 0
                    if dbg:
                        nc.sync.dma_start(dbg_xt, xt8[:])
                        nc.sync.dma_start(dbg_mb, mb[:])
                        nc.sync.dma_start(dbg_xcq, xcq[:])
                        nc.sync.dma_start(dbg_hs, hs[0][:])
                        dxc = dxm_pool.tile([128, 1024], F32, name="dxc")
                        nc.vector.tensor_copy(dxc[:], dxo[:])
                        nc.sync.dma_start(dbg_dx, dxc[:])

                    # ---- mask * dx, residual add (quad layout) ----
                    dxm = dxm_pool.tile([128, 1024], F16, name="dxm")
                    nc.vector.tensor_mul(dxm[:], dxo[:], mb[:])
                    xn = xn_pool.tile([128, 4 * WP], F16, name="xn")
                    xnv = xn[:].rearrange("p (b c) -> p b c", c=WP)
                    nc.gpsimd.memset(xnv[:, :, 0:WP:WP - 1], 0.0)
                    nc.vector.scalar_tensor_tensor(
                        xnv[:, :, 1:W + 1],
                        dxm[:].rearrange("p (b c) -> p b c", c=W), 1.0,
                        xcq[:].rearrange("p (b c) -> p b c", c=WP)[:, :, 1:W + 1],
                        op0=OP.mult, op1=OP.add)
                    if dbg:
                        nc.sync.dma_start(dbg_dxm, dxm[:])
                        nc.sync.dma_start(dbg_xn, xn[:])

                    # ---- stores + next-step hi/lo generation ----
                    # one DMA per quad -> DRAM-side outer dim = 32 channels;
                    # issued on the second HWDGE ring (scalar) to split the
                    # issue load between the two rings
                    def quad_store(dst, src_tile, nelem):
                        for q in range(4):
                            nc.scalar.dma_start(
                                AP(dst.tensor,
                                   dst.offset + s * PL + (r0 + 4 * q) * WP,
                                   [[H * WP, 32], [1, nelem]]),
                                AP(src_tile[:].tensor, 32 * q * nelem,
                                   [[nelem, 32], [1, nelem]]))

                    quad_store(dst16, xn, 4 * WP)
                    if not last:
                        hi8 = hi8_pool.tile([128, 4 * WP], F8, name="hi8")
                        nc.gpsimd.tensor_copy(hi8[:], xn[:])
                        lo8 = lo8_pool.tile([128, 4 * WP], F8, name="lo8")
                        nc.gpsimd.tensor_sub(lo8[:], xn[:], hi8[:])
                        quad_store(dsth, hi8, 4 * WP)
                        quad_store(dstl, lo8, 4 * WP)
    nc.compile()
    return nc


_NC_CACHE = None


def _get_nc():
    global _NC_CACHE
    if _NC_CACHE is None:
        _NC_CACHE = _build()
    return _NC_CACHE


def _q8(x):
    return np.asarray(x, np.float32).astype(NPF8)


def _make_in_maps(x, f1, f2, w1, b1, w2, stoch):
    f1 = np.asarray(f1, np.float64)[:, :, 0, :]   # [3,3,32]
    f2 = np.asarray(f2, np.float64)[:, :, 0, :]
    w1 = np.asarray(w1, np.float64)               # [96,256]
    b1 = np.asarray(b1, np.float64)               # [256]
    w2 = np.asarray(w2, np.float64).copy()        # [256,32]
    w2[:, :IMG] = 0.0                             # ch_mask folded into w2

    # W_eff[dy,dx][c,:] = f1[dy,dx,c]*w1[32+c,:] + f2[dy,dx,c]*w1[64+c,:]
    weff = (f1[:, :, :, None] * w1[None, None, 32:64, :]
            + f2[:, :, :, None] * w1[None, None, 64:96, :])   # [3,3,32,256]
    weff[1, 1] += w1[0:32, :]
    weff = weff.astype(np.float32)

    wc_q = _q8(S1 * weff[1, 1]).astype(np.float32) / S1   # center tap as fp8
    wres = (weff[1, 1] - wc_q).astype(np.float32)          # its residual
    wa = np.zeros((2, 128, 2, 128), np.float32)
    wb = np.zeros((2, 128, 2, 128), np.float32)
    for h in range(2):
        cols = slice(128 * h, 128 * (h + 1))
        for g in range(3):
            rows = slice(32 * g, 32 * (g + 1))
            wa[h, rows, 0, :] = S1 * weff[g, 0, :, cols]
            wa[h, rows, 1, :] = S1 * weff[g, 1, :, cols]
            wb[h, rows, 0, :] = S1 * weff[g, 2, :, cols]
        wa[h, 96:128, 1, :] = S1 * weff[1, 1, :, cols]   # lo through center w
        wb[h, 96:128, 0, :] = S1 * wres[:, cols]         # wres on hi-center
        wb[h, 0, 1, :] = S1 * np.asarray(b1, np.float32)[cols]
    wa = _q8(wa.reshape(2, 128, 256))
    wb = _q8(wb.reshape(2, 128, 256))
    w2a = np.asarray(w2[0:128, :], np.float32).astype(np.float16)
    w2b = np.asarray(w2[128:256, :], np.float32).astype(np.float16)

    x = np.asarray(x, np.float32)
    stoch = np.asarray(stoch, np.float32)
    in_maps = []
    for i in range(NCORES):
        xi = np.transpose(x[i * BPC:(i + 1) * BPC], (0, 3, 1, 2))  # [2,32,H,W]
        xpad = np.zeros((BPC, C, H, WP), np.float32)
        xpad[:, :, :, 1:W + 1] = xi
        x16 = xpad.astype(np.float16)
        x8h = x16.astype(np.float32).astype(NPF8)
        x8l = (x16.astype(np.float32) - x8h.astype(np.float32)).astype(NPF8)
        m8 = _q8((stoch[:, i * BPC:(i + 1) * BPC, :, :, 0] > 0.5))
        in_maps.append({"x16i": x16, "x8hi": x8h, "x8lo": x8l, "mask8": m8,
                        "wa": wa, "wb": wb, "w2a": w2a, "w2b": w2b})
    return in_maps


def kernel(x, f1, f2, w1, b1, w2, stoch, steps):
    assert int(steps) == NSTEP, f"kernel compiled for {NSTEP} steps, got {steps}"
    nc = _get_nc()
    in_maps = _make_in_maps(x, f1, f2, w1, b1, w2, stoch)
    res = run_bass_kernel_spmd(nc, in_maps, core_ids=list(range(NCORES)))
    outs = []
    for i in range(NCORES):
        yi = res.results[i]["y"].astype(np.float32)[:, :, :, 1:W + 1]
        outs.append(np.transpose(yi, (0, 2, 3, 1)))   # -> [2,256,256,32]
    return np.ascontiguousarray(np.concatenate(outs, axis=0)).astype(np.float32)

